# revision 25
# baseline (speedup 1.0000x reference)
"""DepthAttention Trainium2 Bass kernel.

Computes, per (b, h) pair:
    scores = (Q @ K^T) / sqrt(D) * sigmoid(depth)
    scores = where(mask, -inf, scores)
    p_attn = softmax(scores, axis=-1)
    p_val  = p_attn @ V

Sharding: the 32 (b*h) pairs are split 4-per-core across 8 NeuronCores
(data/head parallel, no cross-core communication).

Per-core pipeline (per pair, per 128-row q-tile):
    PE  : S = Q K^T  in float32r (1 cycle/row vs 4 for fp32)
    ACT : t = tanh(depth/2)            [sigmoid via tanh: same ACT table
          e = exp(g/16) + rowsum        set as exp -> no table switches]
    DVE : g = (t + 1) * S              [fused scalar_tensor_tensor]
    DMA : g += maskneg (i16 -30000 where masked, cast+accumulate DMA)
    PE  : Et = transpose(e); p_valT = V^T @ Et (PSUM accumulate)
    GPS : p_attn = e * (1/rowsum)
    DVE : Et PSUM->SBUF copies; p_val = transpose(p_valT) * (1/rowsum)
"""

import numpy as np
from contextlib import ExitStack

import concourse.bass as bass
import concourse.bacc as bacc
import concourse.tile as tile
from concourse import mybir
from concourse.masks import make_identity
from concourse.bass_utils import run_bass_kernel_spmd

B, H, S, D = 4, 8, 1024, 64
N_CORES = 8
PAIRS = (B * H) // N_CORES  # 4 pairs per core
P = 128                     # partitions
QT = S // P                 # 8 q-tiles (and k-tiles) of 128 per pair
F32 = mybir.dt.float32
F32R = mybir.dt.float32r
I16 = mybir.dt.int16
AF = mybir.ActivationFunctionType
OP = mybir.AluOpType

# exp((g - 30000)/16) == 0.0 exactly in fp32 for any unmasked g
MASK_NEG = -30000

# Tunables (validated by local walrus compile + CoreSim before HW):
MASK_VIA_DMA_ACCUM = True   # mask add via SWDGE cast+accum DMA (else DVE copy_predicated)
PA_NORM_ON_GPSIMD = False   # GpSimd TENSOR_SCALAR measured 15.7us/op -> keep on DVE
F32R_TRANSPOSES = True      # exp output (and Q/K loads) in f32r -> 1.5 cyc/row transposes


def build_bass() -> bass.Bass:
    nc = bacc.Bacc(None)

    q_d = nc.dram_tensor("query", [PAIRS, S, D], F32, kind="ExternalInput")
    k_d = nc.dram_tensor("key", [PAIRS, S, D], F32, kind="ExternalInput")
    v_d = nc.dram_tensor("value", [PAIRS, S, D], F32, kind="ExternalInput")
    m_d = nc.dram_tensor("mask", [PAIRS, S, S], mybir.dt.uint8, kind="ExternalInput")
    d_d = nc.dram_tensor("depth", [PAIRS, S, S], F32, kind="ExternalInput")
    pv_d = nc.dram_tensor("p_val", [PAIRS, S, D], F32, kind="ExternalOutput")
    pa_d = nc.dram_tensor("p_attn", [PAIRS, S, S], F32, kind="ExternalOutput")

    QKT = F32R if F32R_TRANSPOSES else F32
    ET = F32R if F32R_TRANSPOSES else F32

    with tile.TileContext(nc) as tc, ExitStack() as ctx:
        consts = ctx.enter_context(tc.tile_pool(name="consts", bufs=1))
        qkv = ctx.enter_context(tc.tile_pool(name="qkv", bufs=2))
        qtkt = ctx.enter_context(tc.tile_pool(name="qtkt", bufs=2))
        dm = ctx.enter_context(tc.tile_pool(name="dm", bufs=2))
        work = ctx.enter_context(tc.tile_pool(name="work", bufs=3))
        pa_pool = ctx.enter_context(tc.tile_pool(name="pa", bufs=2))
        pt_pool = ctx.enter_context(tc.tile_pool(name="pt", bufs=2))
        small = ctx.enter_context(tc.tile_pool(name="small", bufs=2))
        psum_s = ctx.enter_context(
            tc.tile_pool(name="psum_s", bufs=2, space=bass.MemorySpace.PSUM)
        )
        psum_t = ctx.enter_context(
            tc.tile_pool(name="psum_t", bufs=2, space=bass.MemorySpace.PSUM)
        )
        psum_v = ctx.enter_context(
            tc.tile_pool(name="psum_v", bufs=2, space=bass.MemorySpace.PSUM)
        )

        identity = consts.tile([P, P], F32, tag="ident")
        make_identity(nc, identity[:])
        negbig = consts.tile([P, S], F32, tag="negbig")
        nc.vector.memset(negbig[:], float(MASK_NEG))
        if F32R_TRANSPOSES:
            identity_r = consts.tile([P, P], F32R, tag="identr")
            nc.vector.tensor_copy(identity_r[:], identity[:])
        else:
            identity_r = identity

        for h in range(PAIRS):
            # ---------- load Q, K, V as [128, 8, 64] (cast to f32r) ----------
            q_sb = qkv.tile([P, QT, D], QKT, tag="q")
            k_sb = qkv.tile([P, QT, D], QKT, tag="k")
            v_sb = qkv.tile([P, QT, D], F32R, tag="v")
            qv = q_d[h].rearrange("(t p) d -> p t d", p=P)
            kv = k_d[h].rearrange("(t p) d -> p t d", p=P)
            vv = v_d[h].rearrange("(t p) d -> p t d", p=P)
            if F32R_TRANSPOSES:
                nc.gpsimd.dma_start(q_sb[:], qv)
                nc.gpsimd.dma_start(k_sb[:], kv)
            else:
                nc.sync.dma_start(q_sb[:], qv)
                nc.sync.dma_start(k_sb[:], kv)
            nc.gpsimd.dma_start(v_sb[:], vv)

            # ---------- build Qt [64, 1024], Kt [64, 1024] via PE transpose ----
            qt_sb = qtkt.tile([D, S], F32R, tag="qt")
            kt_sb = qtkt.tile([D, S], F32R, tag="kt")
            for src, dst in ((q_sb, qt_sb), (k_sb, kt_sb)):
                for g in range(2):
                    tp = psum_t.tile([P, 512], QKT, tag="tp", name="tp")
                    for j in range(4):
                        ti = g * 4 + j
                        nc.tensor.transpose(
                            tp[0:D, j * P : (j + 1) * P], src[:, ti, :], identity_r[:]
                        )
                    nc.scalar.copy(dst[:, g * 512 : (g + 1) * 512], tp[0:D, :])

            rowsum = small.tile([P, QT], F32, tag="rowsum")
            recip = small.tile([P, QT], F32, tag="recip")
            pval_sb = small.tile([P, QT, D], F32, tag="pval")

            dep_v = d_d[h].rearrange("(hf t p) k -> hf p t k", hf=2, p=P)
            msk_v = m_d[h].rearrange("(hf t p) k -> hf p t k", hf=2, p=P)
            pa_v = pa_d[h].rearrange("(hf t p) k -> hf p t k", hf=2, p=P)

            for half in range(2):
                dep = dm.tile([P, 4, S], F32, tag="dep")
                nc.sync.dma_start(dep[:], dep_v[half])
                msk = dm.tile([P, 4, S], mybir.dt.uint8, tag="msk")
                nc.sync.dma_start(msk[:], msk_v[half])

                # t = tanh(depth / 2) for the whole half
                t_half = dm.tile([P, 4, S], F32, tag="th")
                nc.scalar.activation(t_half[:], dep[:], AF.Tanh, scale=0.5)

                pt_sb = pt_pool.tile([P, QT, 512], ET, tag="ptsb")
                pa_half = pa_pool.tile([P, 4, S], F32, tag="pah")

                for ql in range(4):
                    qi = half * 4 + ql
                    # scores = Q K^T (raw; the 1/sqrt(D) is folded into exp)
                    s_ps = psum_s.tile([P, S], F32, tag="s", name="s_ps")
                    for n in range(2):
                        nc.tensor.matmul(
                            s_ps[:, n * 512 : (n + 1) * 512],
                            qt_sb[:, qi * P : (qi + 1) * P],
                            kt_sb[:, n * 512 : (n + 1) * 512],
                        )
                    # g = (t + 1) * S ; masked entries forced to MASK_NEG
                    g_sb = work.tile([P, S], F32, tag="g", name="g_sb")
                    nc.vector.scalar_tensor_tensor(
                        g_sb[:],
                        t_half[:, ql, :],
                        1.0,
                        s_ps[:],
                        op0=OP.add,
                        op1=OP.mult,
                    )
                    nc.vector.copy_predicated(g_sb[:], msk[:, ql, :], negbig[:])

                    # e = exp(g/16), rowsum = sum_k e
                    e_sb = work.tile([P, S], ET, tag="e", name="e_sb")
                    nc.scalar.activation(
                        e_sb[:],
                        g_sb[:],
                        AF.Exp,
                        scale=1.0 / 16.0,
                        accum_out=rowsum[:, qi : qi + 1],
                    )
                    nc.vector.reciprocal(recip[:, qi : qi + 1], rowsum[:, qi : qi + 1])

                    # p_attn tile = e * recip
                    nc.vector.tensor_scalar_mul(
                        pa_half[:, ql, :], e_sb[:], recip[:, qi : qi + 1]
                    )

                    # Et tiles (transpose e) -> pt_sb [128k, kj, 128q-block]
                    for kg in range(2):
                        tp = psum_t.tile([P, 512], ET, tag="tp", name="tp")
                        for j in range(4):
                            kj = kg * 4 + j
                            nc.tensor.transpose(
                                tp[:, j * P : (j + 1) * P],
                                e_sb[:, kj * P : (kj + 1) * P],
                                identity_r[:],
                            )
                        copy_dst = pt_sb[
                            :, kg * 4 : (kg + 1) * 4, ql * P : (ql + 1) * P
                        ]
                        copy_src = tp[:].rearrange("p (a b) -> p a b", a=4)
                        if kg == 0:
                            nc.vector.tensor_copy(copy_dst, copy_src)
                        else:
                            nc.scalar.copy(copy_dst, copy_src)

                # store p_attn for this half (2 MB DMA)
                nc.sync.dma_start(pa_v[half], pa_half[:])

                # p_valT[64, 512] = sum_kj V[kj]^T @ Et[kj]   (unnormalized)
                pv_ps = psum_v.tile([P, 512], F32, tag="pv", name="pv_ps")
                for kj in range(QT):
                    nc.tensor.matmul(
                        pv_ps[0:D, :],
                        v_sb[:, kj, :],
                        pt_sb[:, kj, :],
                        start=(kj == 0),
                        stop=(kj == QT - 1),
                    )
                pvt = small.tile([D, 512], F32, tag="pvt")
                nc.scalar.copy(pvt[:], pv_ps[0:D, :])

                # transpose back to [128q, 64d] and normalize
                pv2 = psum_v.tile([P, 512], F32, tag="pv", name="pv2")
                for ql in range(4):
                    nc.tensor.transpose(
                        pv2[:, ql * D : (ql + 1) * D],
                        pvt[:, ql * P : (ql + 1) * P],
                        identity[0:D, 0:D],
                    )
                for ql in range(4):
                    qi = half * 4 + ql
                    nc.vector.tensor_scalar_mul(
                        pval_sb[:, qi, :],
                        pv2[:, ql * D : (ql + 1) * D],
                        recip[:, qi : qi + 1],
                    )

            nc.sync.dma_start(pv_d[h].rearrange("(t p) d -> p t d", p=P), pval_sb[:])

    nc.compile()
    return nc


_NC_CACHE = None


def _get_nc() -> bass.Bass:
    global _NC_CACHE
    if _NC_CACHE is None:
        _NC_CACHE = build_bass()
    return _NC_CACHE


def _shard_inputs(inputs):
    q = np.asarray(inputs["query"], dtype=np.float32).reshape(B * H, S, D)
    k = np.asarray(inputs["key"], dtype=np.float32).reshape(B * H, S, D)
    v = np.asarray(inputs["value"], dtype=np.float32).reshape(B * H, S, D)
    m = np.asarray(inputs["mask"]).reshape(B * H, S, S).astype(np.uint8)
    dep = np.asarray(inputs["depth"], dtype=np.float32).reshape(B * H, S, S)
    in_maps = []
    for c in range(N_CORES):
        sl = slice(c * PAIRS, (c + 1) * PAIRS)
        in_maps.append(
            {
                "query": np.ascontiguousarray(q[sl]),
                "key": np.ascontiguousarray(k[sl]),
                "value": np.ascontiguousarray(v[sl]),
                "mask": np.ascontiguousarray(m[sl]),
                "depth": np.ascontiguousarray(dep[sl]),
            }
        )
    return in_maps


def run_sharded(inputs, trace: bool = False, **kwargs):
    """Run the SPMD kernel on 8 cores; returns ((p_val, p_attn), BassKernelResults)."""
    in_maps = _shard_inputs(inputs)
    nc = _get_nc()
    res = run_bass_kernel_spmd(
        nc, in_maps, core_ids=list(range(N_CORES)), trace=trace, **kwargs
    )
    pv = np.concatenate([r["p_val"] for r in res.results], axis=0).reshape(B, H, S, D)
    pa = np.concatenate([r["p_attn"] for r in res.results], axis=0).reshape(B, H, S, S)
    return (pv, pa), res


def kernel(**inputs):
    (pv, pa), _ = run_sharded(inputs)
    return pv, pa
